# revision 27
# baseline (speedup 1.0000x reference)
"""Tensor-parallel GQA multi-head attention for 8 Trainium2 NeuronCores.

Sharding: query heads (16) split 2-per-core; each core needs exactly one
KV head (GQA group); wq/wk/wv column-parallel, wo row-parallel; the
all-reduce after wo is done host-side (sum of 8 bf16 partial outputs).

Single software-pipelined instruction stream per core:
  - All data bf16 (same PE rate as f32r in the cost model, half the DMA).
  - Projection blocks (512 tokens) interleaved with attention groups as
    their Q/K/V become ready, so the PE never drains between phases.
  - Q/K transposed layout [dh, tok]; V projected directly into natural
    layout [tok_local, dh] chunks (lhsT = x-chunk) - no PE transposes.
  - Causal mask: exp first, then gpsimd affine_select zeroes the upper
    triangle of diagonal 128-blocks of P^T (scores are bounded, no
    max-subtraction needed).
  - Softmax denominators: near-free [128,1]-output matmuls
    (lhsT=pt-chunk, rhs=ones) accumulated per token-chunk in PSUM;
    reciprocal on DVE; transposed to rows via one tiny PE transpose;
    broadcast with outer-product matmuls; applied in the AVT copy.
  - Output projection: per 128-token x 512-feature unit, 2 head-matmuls
    accumulated in PSUM, copied+converted to bf16 round-robin over
    DVE/Act/Pool, DMA'd out per 2048-feature row block.
"""

import numpy as np

B, T, D, H, KV = 2, 2048, 2048, 16, 4
DH = 128
NCORES = 8
HPC = H // NCORES          # 2 query heads per core
BT = B * T                 # 4096
ND = D // 128              # 16 contraction chunks
NBLK = BT // 512           # 8 projection blocks
ROPE_BASE = 10000.0

_cache = {}


def _ensure_path():
    try:
        import concourse.bass  # noqa: F401
    except ImportError:
        import sys
        for p in ("/opt/trn_rl_repo", "/root/.axon_site/_ro/trn_rl_repo"):
            if p not in sys.path:
                sys.path.insert(0, p)
        import concourse.bass  # noqa: F401


def _split_multi_waits(nc, mybir, max_waits=1):
    """This container's walrus rejects >1 sync-wait on one instruction
    (seen on the Tile tail drain). Move extra waits onto preceding NoOps
    on the same engine; per-engine program order preserves semantics."""
    for bb in nc.main_func.blocks:
        new_insts = []
        for ins in bb.instructions:
            si = getattr(ins, "sync_info", None)
            if si is not None and si.on_wait and len(si.on_wait) > max_waits:
                waits = list(si.on_wait)
                extra, keep = waits[:-max_waits], waits[-max_waits:]
                for w in extra:
                    new_insts.append(
                        mybir.InstNoOp(
                            name=nc.get_next_instruction_name(),
                            sync_info=mybir.SyncInfo(on_wait=[w], on_update=[]),
                            bass_nofuse=True,
                            engine=ins.engine,
                            ins=[],
                            outs=[],
                        )
                    )
                si.on_wait = keep
            new_insts.append(ins)
        bb.instructions = new_insts


def _build(split_waits=True):
    _ensure_path()
    import concourse.bass as bass
    import concourse.mybir as mybir
    import concourse.tile as tile
    from concourse.masks import make_identity

    f32 = mybir.dt.float32
    bf16 = mybir.dt.bfloat16
    fp8 = mybir.dt.float8e4
    nc = bass.Bass()

    xT = nc.declare_dram_parameter("xT", [D, BT], bf16, isOutput=False)
    wqT = nc.declare_dram_parameter("wqT", [D, HPC * DH], bf16, isOutput=False)
    wkvT = nc.declare_dram_parameter("wkvT", [D, 2 * DH], bf16, isOutput=False)
    woT = nc.declare_dram_parameter("woT", [HPC * DH, D], bf16, isOutput=False)
    cosT = nc.declare_dram_parameter("cosT", [DH, T], bf16, isOutput=False)
    sinT = nc.declare_dram_parameter("sinT", [DH, T], bf16, isOutput=False)
    rotMT = nc.declare_dram_parameter("rotMT", [DH, DH], bf16, isOutput=False)
    out = nc.declare_dram_parameter("out", [BT, D], bf16, isOutput=True)

    with nc.allow_low_precision(reason="bf16 fast path"), \
         tile.TileContext(nc) as tc:
        with tc.tile_pool(name="persist", bufs=1) as P, \
             tc.tile_pool(name="xp", bufs=3) as XP, \
             tc.tile_pool(name="ptp", bufs=16) as PTP, \
             tc.tile_pool(name="rtp", bufs=2) as RTP, \
             tc.tile_pool(name="rrp", bufs=2) as RRP, \
             tc.tile_pool(name="lrp", bufs=2) as LRP, \
             tc.tile_pool(name="osbp", bufs=2) as OSBP, \
             tc.tile_pool(name="psPJ", bufs=2, space="PSUM") as PPJ, \
             tc.tile_pool(name="psST", bufs=2, space="PSUM") as PST, \
             tc.tile_pool(name="psAV", bufs=1, space="PSUM") as PAV, \
             tc.tile_pool(name="psLT", bufs=1, space="PSUM") as PLT, \
             tc.tile_pool(name="psWO", bufs=2, space="PSUM") as PWO:

            ident_f = P.tile([128, 128], f32, tag="identf")
            make_identity(nc, ident_f[:])
            warm_sb = P.tile([128, 128], bf16, tag="warm")
            nc.gpsimd.memset(warm_sb[:], 0.125)
            ones_col = P.tile([128, 1], bf16, tag="ones_c")
            nc.gpsimd.memset(ones_col[:], 1.0)
            ones_row = P.tile([1, 128], bf16, tag="ones_r")
            nc.gpsimd.memset(ones_row[:], 1.0)

            cos_sb = P.tile([128, T], bf16, tag="cos")
            sin_sb = P.tile([128, T], bf16, tag="sin")
            wq_sb = P.tile([128, ND * HPC * DH], bf16, tag="wq")
            wkv_sb = P.tile([128, ND * 2 * DH], bf16, tag="wkv")
            wo_sb = P.tile([128, HPC * D], bf16, tag="wo")

            QT = [P.tile([128, BT], bf16, tag=f"qt{h}", name=f"qt{h}")
                  for h in range(HPC)]
            KT = P.tile([128, BT], bf16, tag="kt")
            Vn = P.tile([128, BT], bf16, tag="vn")
            AVT8 = P.tile([128, HPC * BT], bf16, tag="avt8")

            # ---------------- DMA helpers ----------------
            def load_wq_quarter(qi):
                lo, hi = qi * (ND // 4), (qi + 1) * (ND // 4)
                nc.sync.dma_start(
                    out=wq_sb[:, lo * 256: hi * 256].rearrange(
                        "p (c m) -> p c m", c=hi - lo),
                    in_=wqT[lo * 128: hi * 128, :].rearrange(
                        "(c p) m -> p c m", p=128))

            def load_x_block(j, quarter=None):
                if quarter is not None:
                    lo, hi = quarter * 4, (quarter + 1) * 4
                    nc.sync.dma_start(
                        out=x0_tiles[quarter][:].rearrange(
                            "p (c m) -> p c m", c=4),
                        in_=xT[lo * 128: hi * 128,
                               j * 512:(j + 1) * 512].rearrange(
                            "(c p) m -> p c m", p=128))
                    return
                xt = x_tiles[j % 3]
                nc.sync.dma_start(
                    out=xt[:].rearrange("p (c m) -> p c m", c=ND),
                    in_=xT[:, j * 512:(j + 1) * 512].rearrange(
                        "(c p) m -> p c m", p=128))

            x_tiles = [XP.tile([128, ND * 512], bf16, tag="xt", name=f"xt{i}")
                       for i in range(3)]
            x0_tiles = [XP.tile([128, 4 * 512], bf16, tag=f"xq{i}", bufs=1,
                                name=f"xq{i}") for i in range(4)]
            # x_tiles reused round-robin by block index; block 0 uses four
            # independent quarter tiles so the first matmuls only wait on
            # the first quarter's DMA.

            def x_slice(j, dc):
                if j == 0:
                    return x0_tiles[dc // 4][:, (dc % 4) * 512:
                                             (dc % 4 + 1) * 512]
                return x_tiles[j % 3][:, dc * 512:(dc + 1) * 512]

            def x_slice_nat(j, dc, tj):
                if j == 0:
                    t = x0_tiles[dc // 4]
                    o = (dc % 4) * 512 + tj * 128
                    return t[:, o: o + 128]
                t = x_tiles[j % 3]
                o = dc * 512 + tj * 128
                return t[:, o: o + 128]

            # ---------------- pending output-projection queue ----------------
            pending_wo = []
            copy_rr = {"i": 0}

            def emit_wo_unit(unit, dma_per_dq=False, dve_only=False):
                base, tj, dq, rrT_t, osb_t = unit
                tcx0 = base + tj * 128
                wo_ps = PWO.tile([128, 512], f32, tag="wo", name="wo_ps")
                for h in range(HPC):
                    nc.tensor.matmul(
                        wo_ps[:],
                        lhsT=AVT8[:, h * BT + tcx0: h * BT + tcx0 + 128],
                        rhs=wo_sb[:, h * D + dq * 512: h * D + (dq + 1) * 512],
                        start=(h == 0), stop=(h == HPC - 1))
                dst = osb_t[:, dq * 512:(dq + 1) * 512]
                k = copy_rr["i"] % 16
                copy_rr["i"] += 1
                if k % 2 == 0 and k != 14 and not dve_only:
                    nc.scalar.copy(dst, wo_ps[:])
                else:
                    nc.vector.tensor_copy(dst, wo_ps[:])
                if dma_per_dq:
                    nc.sync.dma_start(
                        out=out[tcx0: tcx0 + 128, dq * 512:(dq + 1) * 512],
                        in_=dst)
                elif dq == 3:
                    nc.sync.dma_start(
                        out=out[tcx0: tcx0 + 128, :], in_=osb_t[:])

            tail_mode = {"on": False}

            def pop_wo(n=1, min_keep=0, dma_per_dq=False):
                for _ in range(n):
                    if len(pending_wo) > min_keep:
                        emit_wo_unit(pending_wo.pop(0), dma_per_dq=dma_per_dq,
                                     dve_only=tail_mode["on"])

            # ---------------- projection block ----------------
            def proj_block(j, min_keep=0):
                tcol = slice(j * 512, (j + 1) * 512)
                tab = slice((j * 512) % T, (j * 512) % T + 512)

                def rope(tgt):
                    rswp = RTP.tile([128, 512], bf16, tag="rs")
                    nc.sync.dma_start(out=rswp[0:64, :], in_=tgt[64:128, tcol])
                    nc.sync.dma_start(out=rswp[64:128, :], in_=tgt[0:64, tcol])
                    rtmp = RTP.tile([128, 512], bf16, tag="rt")
                    nc.vector.tensor_mul(rtmp[:], rswp[:], sin_sb[:, tab])
                    nc.gpsimd.tensor_mul(tgt[:, tcol], tgt[:, tcol],
                                         cos_sb[:, tab])
                    nc.gpsimd.tensor_add(tgt[:, tcol], tgt[:, tcol], rtmp[:])

                # q0, q1 chains
                pq = [PPJ.tile([128, 512], f32, tag="pj", name=f"pq{h}")
                      for h in range(HPC)]
                for dc in range(ND):
                    xs = x_slice(j, dc)
                    st, sp = (dc == 0), (dc == ND - 1)
                    for h in range(HPC):
                        nc.tensor.matmul(
                            pq[h][:],
                            lhsT=wq_sb[:, dc * 256 + h * 128:
                                       dc * 256 + (h + 1) * 128],
                            rhs=xs, start=st, stop=sp)
                    if dc % 4 == 3:
                        pop_wo(min_keep=min_keep)
                for h in range(HPC):
                    nc.vector.tensor_copy(QT[h][:, tcol], pq[h][:])
                for h in range(HPC):
                    rope(QT[h])
                pop_wo(min_keep=min_keep)
                # k chain + v natural chains
                pk = PPJ.tile([128, 512], f32, tag="pj", name="pk")
                pv = PPJ.tile([128, 512], f32, tag="pj", name="pv")
                for dc in range(ND):
                    xs = x_slice(j, dc)
                    st, sp = (dc == 0), (dc == ND - 1)
                    nc.tensor.matmul(
                        pk[:], lhsT=wkv_sb[:, dc * 256: dc * 256 + 128],
                        rhs=xs, start=st, stop=sp)
                    if dc % 4 == 3:
                        pop_wo(min_keep=min_keep)
                nc.vector.tensor_copy(KT[:, tcol], pk[:])
                rope(KT)
                for tj in range(4):
                    for dc in range(ND):
                        nc.tensor.matmul(
                            pv[:, tj * 128:(tj + 1) * 128],
                            lhsT=x_slice_nat(j, dc, tj),
                            rhs=wkv_sb[:, dc * 256 + 128: dc * 256 + 256],
                            start=(dc == 0), stop=(dc == ND - 1))
                    pop_wo(min_keep=min_keep)
                nc.vector.tensor_copy(Vn[:, tcol], pv[:])
                pop_wo(min_keep=min_keep)

            # ---------------- attention group ----------------
            def attn_group(b, tsb):
                n_sc = (tsb + 1) * 4
                base = b * T + tsb * 512
                lt_full = PLT.tile([128, 8], f32, tag="lt", name="lt_ps")
                rrT = RRP.tile([128, 8], f32, tag="rrT")
                osb_tiles = [OSBP.tile([128, D], bf16, tag="osb",
                                       name=f"osb{tj}") for tj in range(4)]
                for h in range(HPC):
                    av_ps = PAV.tile([128, 512], f32, tag="av", name="av_ps")
                    pts = []
                    for sc in range(n_sc):
                        sc_rel = sc - tsb * 4
                        c0 = max(sc_rel, 0) * 128
                        nv = slice(c0, 512)
                        st_ps = PST.tile([128, 512], f32, tag="st",
                                         name="st_ps")
                        nc.tensor.matmul(
                            st_ps[:, nv],
                            lhsT=KT[:, b * T + sc * 128:
                                    b * T + (sc + 1) * 128],
                            rhs=QT[h][:, base + c0: base + 512],
                            start=True, stop=True)
                        pt = PTP.tile([128, 512], bf16, tag="pt")
                        pts.append(pt)
                        nc.scalar.activation(
                            pt[:, nv], st_ps[:, nv],
                            mybir.ActivationFunctionType.Exp)
                        if sc_rel >= 0:
                            blk = pt[:, c0:c0 + 128]
                            nc.gpsimd.affine_select(
                                out=blk, in_=blk,
                                compare_op=mybir.AluOpType.is_ge,
                                fill=0.0, base=0, pattern=[[1, 128]],
                                channel_multiplier=-1)
                        nc.tensor.matmul(
                            av_ps[:, nv],
                            lhsT=Vn[:, b * T + sc * 128:
                                    b * T + (sc + 1) * 128],
                            rhs=pt[:, nv],
                            start=(sc == 0), stop=(sc == n_sc - 1))
                        pop_wo()
                    for tj in range(4):
                        last = tsb * 4 + tj
                        for sc in range(last + 1):
                            nc.tensor.matmul(
                                lt_full[:, h * 4 + tj: h * 4 + tj + 1],
                                lhsT=pts[sc][:, tj * 128:(tj + 1) * 128],
                                rhs=ones_col[:],
                                start=(sc == 0), stop=(sc == last))
                    # normalize: rr = 1/l, transposed to rows, broadcast
                    # via outer product, applied in the AVT copy.
                    nc.vector.reciprocal(rrT[:, h * 4: h * 4 + 4],
                                         lt_full[:, h * 4: h * 4 + 4])
                    tr_ps = PST.tile([128, 512], f32, tag="st", name="tr_ps")
                    for tj in range(4):
                        nc.tensor.transpose(
                            tr_ps[0:1, tj * 128:(tj + 1) * 128],
                            rrT[:, h * 4 + tj: h * 4 + tj + 1], ident_f[:])
                    lrow = LRP.tile([1, 512], bf16, tag="lrow")
                    nc.vector.tensor_copy(lrow[:], tr_ps[0:1, 0:512])
                    rr_ps = PST.tile([128, 512], f32, tag="st", name="rr_ps")
                    for tj in range(4):
                        nc.tensor.matmul(
                            rr_ps[:, tj * 128:(tj + 1) * 128],
                            lhsT=ones_row[:],
                            rhs=lrow[:, tj * 128:(tj + 1) * 128],
                            start=True, stop=True)
                    rr_sb = RTP.tile([128, 512], f32, tag="rrsb", bufs=1)
                    if (b + tsb + h) % 2 == 0:
                        nc.scalar.copy(rr_sb[:], rr_ps[:])
                    else:
                        nc.vector.tensor_copy(rr_sb[:], rr_ps[:])
                    nc.vector.tensor_mul(AVT8[:, h * BT + base: h * BT + base + 512],
                                         av_ps[:], rr_sb[:])
                for tj in range(4):
                    for dq in range(4):
                        pending_wo.append((base, tj, dq, rrT, osb_tiles[tj]))

            # ---------------- emission schedule ----------------
            load_x_block(0, quarter=0)
            load_wq_quarter(0)
            load_x_block(0, quarter=1)
            load_wq_quarter(1)
            load_x_block(0, quarter=2)
            load_x_block(0, quarter=3)
            load_wq_quarter(2)
            load_wq_quarter(3)
            nc.sync.dma_start(
                out=wkv_sb[:].rearrange("p (c m) -> p c m", c=ND),
                in_=wkvT[:, :].rearrange("(c p) m -> p c m", p=128))
            load_x_block(1)
            nc.sync.dma_start(out=cos_sb[:], in_=cosT[:, :])
            nc.sync.dma_start(out=sin_sb[:], in_=sinT[:, :])
            load_x_block(2)
            nc.sync.dma_start(
                out=wo_sb[:].rearrange("p (c n) -> p c n", c=HPC),
                in_=woT[:, :].rearrange("(c p) n -> p c n", p=128))

            # ramp the PE while the first loads land
            for _ in range(26):
                wps = PST.tile([128, 512], f32, tag="st", name="warm_ps")
                nc.tensor.matmul(wps[:, 0:128], lhsT=warm_sb[:],
                                 rhs=warm_sb[:], start=True, stop=True)

            schedule = [
                ("P", 0), ("P", 1), ("A", 0, 0),
                ("P", 2), ("A", 0, 1),
                ("P", 3), ("A", 0, 2),
                ("P", 4), ("A", 0, 3),
                ("P", 5), ("A", 1, 0),
                ("P", 6), ("A", 1, 1), ("A", 1, 2),
                ("P", 7), ("A", 1, 3),
            ]
            for item in schedule:
                if item[0] == "P":
                    j = item[1]
                    if j + 2 < NBLK and j >= 1:
                        load_x_block(j + 2)
                    proj_block(j, min_keep=12 if j == 7 else 0)
                else:
                    attn_group(item[1], item[2])
            pop_wo(len(pending_wo), dma_per_dq=True)

    if split_waits:
        _split_multi_waits(nc, mybir)
    return nc


def _host_inputs(x, wq, wk, wv, wo):
    import ml_dtypes
    bf = ml_dtypes.bfloat16
    f8 = ml_dtypes.float8_e4m3
    xT = np.ascontiguousarray(x.reshape(BT, D).T).astype(bf)
    half = DH // 2
    inv = (1.0 / (ROPE_BASE ** (np.arange(half, dtype=np.float32) / half))
           ).astype(np.float32)
    ang = np.arange(T, dtype=np.float32)[:, None] * inv[None, :]   # (T, 64)
    c = np.cos(ang).T.astype(np.float32)                           # (64, T)
    s = np.sin(ang).T.astype(np.float32)
    cosT = np.ascontiguousarray(np.concatenate([c, c], axis=0)).astype(bf)
    # first half negated: rope rot = [-x2; x1]*sin == [x2; x1]*[-s; s]
    sinT = np.ascontiguousarray(np.concatenate([-s, s], axis=0)).astype(bf)
    rotMT = np.zeros((DH, DH), dtype=np.float32)
    rotMT[np.arange(64), np.arange(64) + 64] = 1.0
    rotMT[np.arange(64) + 64, np.arange(64)] = -1.0
    rotMT = rotMT.astype(bf)
    scale = np.float32(1.0 / np.sqrt(DH))
    in_maps = []
    for core in range(NCORES):
        kvh = core // 2
        wkv = np.concatenate(
            [wk[kvh * DH:(kvh + 1) * DH, :].T,
             wv[kvh * DH:(kvh + 1) * DH, :].T], axis=1)   # (D, 256)
        in_maps.append({
            "xT": xT,
            "wqT": np.ascontiguousarray(
                (wq[core * HPC * DH:(core + 1) * HPC * DH, :] * scale).T
            ).astype(bf),
            "wkvT": np.ascontiguousarray(wkv).astype(bf),
            "woT": np.ascontiguousarray(
                wo[:, core * HPC * DH:(core + 1) * HPC * DH].T).astype(bf),
            "cosT": cosT,
            "sinT": sinT,
            "rotMT": rotMT,
        })
    return in_maps


def kernel(x, wq, wk, wv, wo):
    _ensure_path()
    from concourse.bass_utils import run_bass_kernel_spmd

    x = np.asarray(x, dtype=np.float32)
    wq = np.asarray(wq, dtype=np.float32)
    wk = np.asarray(wk, dtype=np.float32)
    wv = np.asarray(wv, dtype=np.float32)
    wo = np.asarray(wo, dtype=np.float32)

    if "nc" not in _cache:
        _cache["nc"] = _build()
    nc = _cache["nc"]

    in_maps = _host_inputs(x, wq, wk, wv, wo)
    res = run_bass_kernel_spmd(nc, in_maps, list(range(NCORES)))
    acc = res.results[0]["out"].astype(np.float32)
    for cidx in range(1, NCORES):
        acc = acc + res.results[cidx]["out"].astype(np.float32)
    return acc.reshape(B, T, D)


# revision 28
# speedup vs baseline: 1.0680x; 1.0680x over previous
"""Tensor-parallel GQA multi-head attention for 8 Trainium2 NeuronCores.

Sharding: query heads (16) split 2-per-core; each core needs exactly one
KV head (GQA group); wq/wk/wv column-parallel, wo row-parallel; the
all-reduce after wo is done host-side (sum of 8 bf16 partial outputs).

Single software-pipelined instruction stream per core:
  - All data bf16 (same PE rate as f32r in the cost model, half the DMA).
  - Projection blocks (512 tokens) interleaved with attention groups as
    their Q/K/V become ready, so the PE never drains between phases.
  - Q/K transposed layout [dh, tok]; V projected directly into natural
    layout [tok_local, dh] chunks (lhsT = x-chunk) - no PE transposes.
  - Causal mask: exp first, then gpsimd affine_select zeroes the upper
    triangle of diagonal 128-blocks of P^T (scores are bounded, no
    max-subtraction needed).
  - Softmax denominators: near-free [128,1]-output matmuls
    (lhsT=pt-chunk, rhs=ones) accumulated per token-chunk in PSUM;
    reciprocal on DVE; transposed to rows via one tiny PE transpose;
    broadcast with outer-product matmuls; applied in the AVT copy.
  - Output projection: per 128-token x 512-feature unit, 2 head-matmuls
    accumulated in PSUM, copied+converted to bf16 round-robin over
    DVE/Act/Pool, DMA'd out per 2048-feature row block.
"""

import numpy as np

B, T, D, H, KV = 2, 2048, 2048, 16, 4
DH = 128
NCORES = 8
HPC = H // NCORES          # 2 query heads per core
BT = B * T                 # 4096
ND = D // 128              # 16 contraction chunks
NBLK = BT // 512           # 8 projection blocks
ROPE_BASE = 10000.0

_cache = {}


def _ensure_path():
    try:
        import concourse.bass  # noqa: F401
    except ImportError:
        import sys
        for p in ("/opt/trn_rl_repo", "/root/.axon_site/_ro/trn_rl_repo"):
            if p not in sys.path:
                sys.path.insert(0, p)
        import concourse.bass  # noqa: F401


def _split_multi_waits(nc, mybir, max_waits=1):
    """This container's walrus rejects >1 sync-wait on one instruction
    (seen on the Tile tail drain). Move extra waits onto preceding NoOps
    on the same engine; per-engine program order preserves semantics."""
    for bb in nc.main_func.blocks:
        new_insts = []
        for ins in bb.instructions:
            si = getattr(ins, "sync_info", None)
            if si is not None and si.on_wait and len(si.on_wait) > max_waits:
                waits = list(si.on_wait)
                extra, keep = waits[:-max_waits], waits[-max_waits:]
                for w in extra:
                    new_insts.append(
                        mybir.InstNoOp(
                            name=nc.get_next_instruction_name(),
                            sync_info=mybir.SyncInfo(on_wait=[w], on_update=[]),
                            bass_nofuse=True,
                            engine=ins.engine,
                            ins=[],
                            outs=[],
                        )
                    )
                si.on_wait = keep
            new_insts.append(ins)
        bb.instructions = new_insts


def _build(split_waits=True):
    _ensure_path()
    import concourse.bass as bass
    import concourse.mybir as mybir
    import concourse.tile as tile
    from concourse.masks import make_identity

    f32 = mybir.dt.float32
    bf16 = mybir.dt.bfloat16
    fp8 = mybir.dt.float8e4
    nc = bass.Bass()

    xT = nc.declare_dram_parameter("xT", [D, BT], bf16, isOutput=False)
    wqT = nc.declare_dram_parameter("wqT", [D, HPC * DH], bf16, isOutput=False)
    wkvT = nc.declare_dram_parameter("wkvT", [D, 2 * DH], bf16, isOutput=False)
    woT = nc.declare_dram_parameter("woT", [HPC * DH, D], bf16, isOutput=False)
    cosT = nc.declare_dram_parameter("cosT", [DH, T], bf16, isOutput=False)
    sinT = nc.declare_dram_parameter("sinT", [DH, T], bf16, isOutput=False)
    rotMT = nc.declare_dram_parameter("rotMT", [DH, DH], bf16, isOutput=False)
    out = nc.declare_dram_parameter("out", [BT, D], bf16, isOutput=True)

    with nc.allow_low_precision(reason="bf16 fast path"), \
         tile.TileContext(nc) as tc:
        with tc.tile_pool(name="persist", bufs=1) as P, \
             tc.tile_pool(name="xp", bufs=3) as XP, \
             tc.tile_pool(name="ptp", bufs=16) as PTP, \
             tc.tile_pool(name="rtp", bufs=2) as RTP, \
             tc.tile_pool(name="rrp", bufs=2) as RRP, \
             tc.tile_pool(name="lrp", bufs=2) as LRP, \
             tc.tile_pool(name="osbp", bufs=4) as OSBP, \
             tc.tile_pool(name="psPJ", bufs=2, space="PSUM") as PPJ, \
             tc.tile_pool(name="psST", bufs=2, space="PSUM") as PST, \
             tc.tile_pool(name="psAV", bufs=1, space="PSUM") as PAV, \
             tc.tile_pool(name="psLT", bufs=1, space="PSUM") as PLT, \
             tc.tile_pool(name="psWO", bufs=2, space="PSUM") as PWO:

            ident_f = P.tile([128, 128], f32, tag="identf")
            make_identity(nc, ident_f[:])
            warm_sb = P.tile([128, 128], bf16, tag="warm")
            nc.gpsimd.memset(warm_sb[:], 0.125)
            ones_col = P.tile([128, 1], bf16, tag="ones_c")
            nc.gpsimd.memset(ones_col[:], 1.0)
            ones_row = P.tile([1, 128], bf16, tag="ones_r")
            nc.gpsimd.memset(ones_row[:], 1.0)

            cos_sb = P.tile([128, T], bf16, tag="cos")
            sin_sb = P.tile([128, T], bf16, tag="sin")
            wq_sb = P.tile([128, ND * HPC * DH], bf16, tag="wq")
            wkv_sb = P.tile([128, ND * 2 * DH], bf16, tag="wkv")
            wo_sb = P.tile([128, HPC * D], bf16, tag="wo")

            QT = [P.tile([128, BT], bf16, tag=f"qt{h}", name=f"qt{h}")
                  for h in range(HPC)]
            KT = P.tile([128, BT], bf16, tag="kt")
            Vn = P.tile([128, BT], bf16, tag="vn")
            AVT8 = P.tile([128, HPC * BT], bf16, tag="avt8")

            # ---------------- DMA helpers ----------------
            def load_wq_quarter(qi):
                lo, hi = qi * (ND // 4), (qi + 1) * (ND // 4)
                nc.sync.dma_start(
                    out=wq_sb[:, lo * 256: hi * 256].rearrange(
                        "p (c m) -> p c m", c=hi - lo),
                    in_=wqT[lo * 128: hi * 128, :].rearrange(
                        "(c p) m -> p c m", p=128))

            def load_x_block(j, quarter=None):
                if quarter is not None:
                    lo, hi = quarter * 4, (quarter + 1) * 4
                    nc.sync.dma_start(
                        out=x0_tiles[quarter][:].rearrange(
                            "p (c m) -> p c m", c=4),
                        in_=xT[lo * 128: hi * 128,
                               j * 512:(j + 1) * 512].rearrange(
                            "(c p) m -> p c m", p=128))
                    return
                xt = x_tiles[j % 3]
                nc.sync.dma_start(
                    out=xt[:].rearrange("p (c m) -> p c m", c=ND),
                    in_=xT[:, j * 512:(j + 1) * 512].rearrange(
                        "(c p) m -> p c m", p=128))

            x_tiles = [XP.tile([128, ND * 512], bf16, tag="xt", name=f"xt{i}")
                       for i in range(3)]
            x0_tiles = [XP.tile([128, 4 * 512], bf16, tag=f"xq{i}", bufs=1,
                                name=f"xq{i}") for i in range(4)]
            # x_tiles reused round-robin by block index; block 0 uses four
            # independent quarter tiles so the first matmuls only wait on
            # the first quarter's DMA.

            def x_slice(j, dc):
                if j == 0:
                    return x0_tiles[dc // 4][:, (dc % 4) * 512:
                                             (dc % 4 + 1) * 512]
                return x_tiles[j % 3][:, dc * 512:(dc + 1) * 512]

            def x_slice_nat(j, dc, tj):
                if j == 0:
                    t = x0_tiles[dc // 4]
                    o = (dc % 4) * 512 + tj * 128
                    return t[:, o: o + 128]
                t = x_tiles[j % 3]
                o = dc * 512 + tj * 128
                return t[:, o: o + 128]

            # ---------------- pending output-projection queue ----------------
            pending_wo = []
            copy_rr = {"i": 0}

            def emit_wo_unit(unit, dma_per_dq=False, dve_only=False):
                base, tj, dq, rrT_t, osb_t = unit
                tcx0 = base + tj * 128
                wo_ps = PWO.tile([128, 512], f32, tag="wo", name="wo_ps")
                for h in range(HPC):
                    nc.tensor.matmul(
                        wo_ps[:],
                        lhsT=AVT8[:, h * BT + tcx0: h * BT + tcx0 + 128],
                        rhs=wo_sb[:, h * D + dq * 512: h * D + (dq + 1) * 512],
                        start=(h == 0), stop=(h == HPC - 1))
                dst = osb_t[:, dq * 512:(dq + 1) * 512]
                k = copy_rr["i"] % 16
                copy_rr["i"] += 1
                if k % 2 == 0 and k != 14 and not dve_only:
                    nc.scalar.copy(dst, wo_ps[:])
                else:
                    nc.vector.tensor_copy(dst, wo_ps[:])
                if dma_per_dq:
                    nc.sync.dma_start(
                        out=out[tcx0: tcx0 + 128, dq * 512:(dq + 1) * 512],
                        in_=dst)
                elif dq == 3:
                    nc.sync.dma_start(
                        out=out[tcx0: tcx0 + 128, :], in_=osb_t[:])

            tail_mode = {"on": False}

            def pop_wo(n=1, min_keep=0, dma_per_dq=False):
                for _ in range(n):
                    if len(pending_wo) > min_keep:
                        emit_wo_unit(pending_wo.pop(0), dma_per_dq=dma_per_dq,
                                     dve_only=tail_mode["on"])

            # ---------------- projection block ----------------
            def proj_block(j, min_keep=0):
                tcol = slice(j * 512, (j + 1) * 512)
                tab = slice((j * 512) % T, (j * 512) % T + 512)

                def rope(tgt):
                    rswp = RTP.tile([128, 512], bf16, tag="rs")
                    nc.sync.dma_start(out=rswp[0:64, :], in_=tgt[64:128, tcol])
                    nc.sync.dma_start(out=rswp[64:128, :], in_=tgt[0:64, tcol])
                    rtmp = RTP.tile([128, 512], bf16, tag="rt")
                    nc.vector.tensor_mul(rtmp[:], rswp[:], sin_sb[:, tab])
                    nc.gpsimd.tensor_mul(tgt[:, tcol], tgt[:, tcol],
                                         cos_sb[:, tab])
                    nc.gpsimd.tensor_add(tgt[:, tcol], tgt[:, tcol], rtmp[:])

                # q0, q1 chains
                pq = [PPJ.tile([128, 512], f32, tag="pj", name=f"pq{h}")
                      for h in range(HPC)]
                for dc in range(ND):
                    xs = x_slice(j, dc)
                    st, sp = (dc == 0), (dc == ND - 1)
                    for h in range(HPC):
                        nc.tensor.matmul(
                            pq[h][:],
                            lhsT=wq_sb[:, dc * 256 + h * 128:
                                       dc * 256 + (h + 1) * 128],
                            rhs=xs, start=st, stop=sp)
                    if dc % 4 == 3:
                        pop_wo(min_keep=min_keep)
                for h in range(HPC):
                    nc.vector.tensor_copy(QT[h][:, tcol], pq[h][:])
                for h in range(HPC):
                    rope(QT[h])
                pop_wo(min_keep=min_keep)
                # k chain + v natural chains
                pk = PPJ.tile([128, 512], f32, tag="pj", name="pk")
                pv = PPJ.tile([128, 512], f32, tag="pj", name="pv")
                for dc in range(ND):
                    xs = x_slice(j, dc)
                    st, sp = (dc == 0), (dc == ND - 1)
                    nc.tensor.matmul(
                        pk[:], lhsT=wkv_sb[:, dc * 256: dc * 256 + 128],
                        rhs=xs, start=st, stop=sp)
                    if dc % 4 == 3:
                        pop_wo(min_keep=min_keep)
                nc.vector.tensor_copy(KT[:, tcol], pk[:])
                rope(KT)
                for tj in range(4):
                    for dc in range(ND):
                        nc.tensor.matmul(
                            pv[:, tj * 128:(tj + 1) * 128],
                            lhsT=x_slice_nat(j, dc, tj),
                            rhs=wkv_sb[:, dc * 256 + 128: dc * 256 + 256],
                            start=(dc == 0), stop=(dc == ND - 1))
                    pop_wo(min_keep=min_keep)
                nc.vector.tensor_copy(Vn[:, tcol], pv[:])
                pop_wo(min_keep=min_keep)

            # ---------------- attention group ----------------
            def attn_group(b, tsb):
                n_sc = (tsb + 1) * 4
                base = b * T + tsb * 512
                lt_full = PLT.tile([128, 8], f32, tag="lt", name="lt_ps")
                rrT = RRP.tile([128, 8], f32, tag="rrT")
                osb_tiles = [OSBP.tile([128, D], bf16, tag="osb",
                                       name=f"osb{tj}") for tj in range(4)]
                for h in range(HPC):
                    av_ps = PAV.tile([128, 512], f32, tag="av", name="av_ps")
                    pts = []
                    for sc in range(n_sc):
                        sc_rel = sc - tsb * 4
                        c0 = max(sc_rel, 0) * 128
                        nv = slice(c0, 512)
                        st_ps = PST.tile([128, 512], f32, tag="st",
                                         name="st_ps")
                        nc.tensor.matmul(
                            st_ps[:, nv],
                            lhsT=KT[:, b * T + sc * 128:
                                    b * T + (sc + 1) * 128],
                            rhs=QT[h][:, base + c0: base + 512],
                            start=True, stop=True)
                        pt = PTP.tile([128, 512], bf16, tag="pt")
                        pts.append(pt)
                        nc.scalar.activation(
                            pt[:, nv], st_ps[:, nv],
                            mybir.ActivationFunctionType.Exp)
                        if sc_rel >= 0:
                            blk = pt[:, c0:c0 + 128]
                            nc.gpsimd.affine_select(
                                out=blk, in_=blk,
                                compare_op=mybir.AluOpType.is_ge,
                                fill=0.0, base=0, pattern=[[1, 128]],
                                channel_multiplier=-1)
                        nc.tensor.matmul(
                            av_ps[:, nv],
                            lhsT=Vn[:, b * T + sc * 128:
                                    b * T + (sc + 1) * 128],
                            rhs=pt[:, nv],
                            start=(sc == 0), stop=(sc == n_sc - 1))
                        pop_wo()
                    for tj in range(4):
                        last = tsb * 4 + tj
                        for sc in range(last + 1):
                            nc.tensor.matmul(
                                lt_full[:, h * 4 + tj: h * 4 + tj + 1],
                                lhsT=pts[sc][:, tj * 128:(tj + 1) * 128],
                                rhs=ones_col[:],
                                start=(sc == 0), stop=(sc == last))
                    # normalize: rr = 1/l, transposed to rows, broadcast
                    # via outer product, applied in the AVT copy.
                    nc.vector.reciprocal(rrT[:, h * 4: h * 4 + 4],
                                         lt_full[:, h * 4: h * 4 + 4])
                    tr_ps = PST.tile([128, 512], f32, tag="st", name="tr_ps")
                    for tj in range(4):
                        nc.tensor.transpose(
                            tr_ps[0:1, tj * 128:(tj + 1) * 128],
                            rrT[:, h * 4 + tj: h * 4 + tj + 1], ident_f[:])
                    lrow = LRP.tile([1, 512], bf16, tag="lrow")
                    nc.vector.tensor_copy(lrow[:], tr_ps[0:1, 0:512])
                    rr_ps = PST.tile([128, 512], f32, tag="st", name="rr_ps")
                    for tj in range(4):
                        nc.tensor.matmul(
                            rr_ps[:, tj * 128:(tj + 1) * 128],
                            lhsT=ones_row[:],
                            rhs=lrow[0:1, tj * 128:(tj + 1) * 128],
                            start=True, stop=True)
                    rr_sb = RTP.tile([128, 512], f32, tag="rrsb")
                    if (b + tsb + h) % 2 == 0:
                        nc.scalar.copy(rr_sb[:], rr_ps[:])
                    else:
                        nc.vector.tensor_copy(rr_sb[:], rr_ps[:])
                    nc.vector.tensor_mul(AVT8[:, h * BT + base: h * BT + base + 512],
                                         av_ps[:], rr_sb[:])
                for tj in range(4):
                    for dq in range(4):
                        pending_wo.append((base, tj, dq, rrT, osb_tiles[tj]))

            # ---------------- emission schedule ----------------
            load_x_block(0, quarter=0)
            load_wq_quarter(0)
            load_x_block(0, quarter=1)
            load_wq_quarter(1)
            load_x_block(0, quarter=2)
            load_x_block(0, quarter=3)
            load_wq_quarter(2)
            load_wq_quarter(3)
            nc.sync.dma_start(
                out=wkv_sb[:].rearrange("p (c m) -> p c m", c=ND),
                in_=wkvT[:, :].rearrange("(c p) m -> p c m", p=128))
            load_x_block(1)
            nc.sync.dma_start(out=cos_sb[:], in_=cosT[:, :])
            nc.sync.dma_start(out=sin_sb[:], in_=sinT[:, :])
            load_x_block(2)
            nc.sync.dma_start(
                out=wo_sb[:].rearrange("p (c n) -> p c n", c=HPC),
                in_=woT[:, :].rearrange("(c p) n -> p c n", p=128))

            # ramp the PE while the first loads land
            for _ in range(26):
                wps = PST.tile([128, 512], f32, tag="st", name="warm_ps")
                nc.tensor.matmul(wps[:, 0:128], lhsT=warm_sb[:],
                                 rhs=warm_sb[:], start=True, stop=True)

            schedule = [
                ("P", 0), ("P", 1), ("A", 0, 0),
                ("P", 2), ("A", 0, 1),
                ("P", 3), ("A", 0, 2),
                ("P", 4), ("A", 0, 3),
                ("P", 5), ("A", 1, 0),
                ("P", 6), ("A", 1, 1), ("A", 1, 2),
                ("P", 7), ("A", 1, 3),
            ]
            for item in schedule:
                if item[0] == "P":
                    j = item[1]
                    if j + 2 < NBLK and j > 0:
                        load_x_block(j + 2)
                    proj_block(j, min_keep=12 if j == 7 else 0)
                else:
                    attn_group(item[1], item[2])
            pop_wo(len(pending_wo), dma_per_dq=True)

    if split_waits:
        _split_multi_waits(nc, mybir)
    return nc


def _host_inputs(x, wq, wk, wv, wo):
    import ml_dtypes
    bf = ml_dtypes.bfloat16
    f8 = ml_dtypes.float8_e4m3
    xT = np.ascontiguousarray(x.reshape(BT, D).T).astype(bf)
    half = DH // 2
    inv = (1.0 / (ROPE_BASE ** (np.arange(half, dtype=np.float32) / half))
           ).astype(np.float32)
    ang = np.arange(T, dtype=np.float32)[:, None] * inv[None, :]   # (T, 64)
    c = np.cos(ang).T.astype(np.float32)                           # (64, T)
    s = np.sin(ang).T.astype(np.float32)
    cosT = np.ascontiguousarray(np.concatenate([c, c], axis=0)).astype(bf)
    # first half negated: rope rot = [-x2; x1]*sin == [x2; x1]*[-s; s]
    sinT = np.ascontiguousarray(np.concatenate([-s, s], axis=0)).astype(bf)
    rotMT = np.zeros((DH, DH), dtype=np.float32)
    rotMT[np.arange(64), np.arange(64) + 64] = 1.0
    rotMT[np.arange(64) + 64, np.arange(64)] = -1.0
    rotMT = rotMT.astype(bf)
    scale = np.float32(1.0 / np.sqrt(DH))
    in_maps = []
    for core in range(NCORES):
        kvh = core // 2
        wkv = np.concatenate(
            [wk[kvh * DH:(kvh + 1) * DH, :].T,
             wv[kvh * DH:(kvh + 1) * DH, :].T], axis=1)   # (D, 256)
        in_maps.append({
            "xT": xT,
            "wqT": np.ascontiguousarray(
                (wq[core * HPC * DH:(core + 1) * HPC * DH, :] * scale).T
            ).astype(bf),
            "wkvT": np.ascontiguousarray(wkv).astype(bf),
            "woT": np.ascontiguousarray(
                wo[:, core * HPC * DH:(core + 1) * HPC * DH].T).astype(bf),
            "cosT": cosT,
            "sinT": sinT,
            "rotMT": rotMT,
        })
    return in_maps


def kernel(x, wq, wk, wv, wo):
    _ensure_path()
    from concourse.bass_utils import run_bass_kernel_spmd

    x = np.asarray(x, dtype=np.float32)
    wq = np.asarray(wq, dtype=np.float32)
    wk = np.asarray(wk, dtype=np.float32)
    wv = np.asarray(wv, dtype=np.float32)
    wo = np.asarray(wo, dtype=np.float32)

    if "nc" not in _cache:
        _cache["nc"] = _build()
    nc = _cache["nc"]

    in_maps = _host_inputs(x, wq, wk, wv, wo)
    res = run_bass_kernel_spmd(nc, in_maps, list(range(NCORES)))
    acc = res.results[0]["out"].astype(np.float32)
    for cidx in range(1, NCORES):
        acc = acc + res.results[cidx]["out"].astype(np.float32)
    return acc.reshape(B, T, D)


# revision 29
# speedup vs baseline: 1.0820x; 1.0130x over previous
"""Tensor-parallel GQA multi-head attention for 8 Trainium2 NeuronCores.

Sharding: query heads (16) split 2-per-core; each core needs exactly one
KV head (GQA group); wq/wk/wv column-parallel, wo row-parallel; the
all-reduce after wo is done host-side (sum of 8 bf16 partial outputs).

Single software-pipelined instruction stream per core:
  - All data bf16 (same PE rate as f32r in the cost model, half the DMA).
  - Projection blocks (512 tokens) interleaved with attention groups as
    their Q/K/V become ready, so the PE never drains between phases.
  - Q/K transposed layout [dh, tok]; V projected directly into natural
    layout [tok_local, dh] chunks (lhsT = x-chunk) - no PE transposes.
  - Causal mask: exp first, then gpsimd affine_select zeroes the upper
    triangle of diagonal 128-blocks of P^T (scores are bounded, no
    max-subtraction needed).
  - Softmax denominators: near-free [128,1]-output matmuls
    (lhsT=pt-chunk, rhs=ones) accumulated per token-chunk in PSUM;
    reciprocal on DVE; transposed to rows via one tiny PE transpose;
    broadcast with outer-product matmuls; applied in the AVT copy.
  - Output projection: per 128-token x 512-feature unit, 2 head-matmuls
    accumulated in PSUM, copied+converted to bf16 round-robin over
    DVE/Act/Pool, DMA'd out per 2048-feature row block.
"""

import numpy as np

B, T, D, H, KV = 2, 2048, 2048, 16, 4
DH = 128
NCORES = 8
HPC = H // NCORES          # 2 query heads per core
BT = B * T                 # 4096
ND = D // 128              # 16 contraction chunks
NBLK = BT // 512           # 8 projection blocks
ROPE_BASE = 10000.0

_cache = {}


def _ensure_path():
    try:
        import concourse.bass  # noqa: F401
    except ImportError:
        import sys
        for p in ("/opt/trn_rl_repo", "/root/.axon_site/_ro/trn_rl_repo"):
            if p not in sys.path:
                sys.path.insert(0, p)
        import concourse.bass  # noqa: F401


def _split_multi_waits(nc, mybir, max_waits=1):
    """This container's walrus rejects >1 sync-wait on one instruction
    (seen on the Tile tail drain). Move extra waits onto preceding NoOps
    on the same engine; per-engine program order preserves semantics."""
    for bb in nc.main_func.blocks:
        new_insts = []
        for ins in bb.instructions:
            si = getattr(ins, "sync_info", None)
            if si is not None and si.on_wait and len(si.on_wait) > max_waits:
                waits = list(si.on_wait)
                extra, keep = waits[:-max_waits], waits[-max_waits:]
                for w in extra:
                    new_insts.append(
                        mybir.InstNoOp(
                            name=nc.get_next_instruction_name(),
                            sync_info=mybir.SyncInfo(on_wait=[w], on_update=[]),
                            bass_nofuse=True,
                            engine=ins.engine,
                            ins=[],
                            outs=[],
                        )
                    )
                si.on_wait = keep
            new_insts.append(ins)
        bb.instructions = new_insts


def _build(split_waits=True):
    _ensure_path()
    import concourse.bass as bass
    import concourse.mybir as mybir
    import concourse.tile as tile
    from concourse.masks import make_identity

    f32 = mybir.dt.float32
    bf16 = mybir.dt.bfloat16
    fp8 = mybir.dt.float8e4
    nc = bass.Bass()

    xT = nc.declare_dram_parameter("xT", [D, BT], bf16, isOutput=False)
    wqT = nc.declare_dram_parameter("wqT", [D, HPC * DH], bf16, isOutput=False)
    wkvT = nc.declare_dram_parameter("wkvT", [D, 2 * DH], bf16, isOutput=False)
    woT = nc.declare_dram_parameter("woT", [HPC * DH, D], bf16, isOutput=False)
    cosT = nc.declare_dram_parameter("cosT", [DH, T], bf16, isOutput=False)
    sinT = nc.declare_dram_parameter("sinT", [DH, T], bf16, isOutput=False)
    rotMT = nc.declare_dram_parameter("rotMT", [DH, DH], bf16, isOutput=False)
    out = nc.declare_dram_parameter("out", [BT, D], bf16, isOutput=True)

    with nc.allow_low_precision(reason="bf16 fast path"), \
         tile.TileContext(nc) as tc:
        with tc.tile_pool(name="persist", bufs=1) as P, \
             tc.tile_pool(name="xp", bufs=3) as XP, \
             tc.tile_pool(name="ptp", bufs=16) as PTP, \
             tc.tile_pool(name="rtp", bufs=2) as RTP, \
             tc.tile_pool(name="rrp", bufs=2) as RRP, \
             tc.tile_pool(name="lrp", bufs=2) as LRP, \
             tc.tile_pool(name="osbp", bufs=4) as OSBP, \
             tc.tile_pool(name="psPJ", bufs=2, space="PSUM") as PPJ, \
             tc.tile_pool(name="psST", bufs=2, space="PSUM") as PST, \
             tc.tile_pool(name="psAV", bufs=1, space="PSUM") as PAV, \
             tc.tile_pool(name="psLT", bufs=1, space="PSUM") as PLT, \
             tc.tile_pool(name="psWO", bufs=2, space="PSUM") as PWO:

            ident_f = P.tile([128, 128], f32, tag="identf")
            make_identity(nc, ident_f[:])
            warm_sb = P.tile([128, 128], bf16, tag="warm")
            nc.gpsimd.memset(warm_sb[:], 0.125)
            ones_col = P.tile([128, 1], bf16, tag="ones_c")
            nc.gpsimd.memset(ones_col[:], 1.0)
            ones_row = P.tile([1, 128], bf16, tag="ones_r")
            nc.gpsimd.memset(ones_row[:], 1.0)

            cos_sb = P.tile([128, T], bf16, tag="cos")
            sin_sb = P.tile([128, T], bf16, tag="sin")
            wq_sb = P.tile([128, ND * HPC * DH], bf16, tag="wq")
            wkv_sb = P.tile([128, ND * 2 * DH], bf16, tag="wkv")
            wo_sb = P.tile([128, HPC * D], bf16, tag="wo")

            QT = [P.tile([128, BT], bf16, tag=f"qt{h}", name=f"qt{h}")
                  for h in range(HPC)]
            KT = P.tile([128, BT], bf16, tag="kt")
            Vn = P.tile([128, BT], bf16, tag="vn")
            AVT8 = P.tile([128, HPC * BT], bf16, tag="avt8")

            # ---------------- DMA helpers ----------------
            def load_wq_quarter(qi):
                lo, hi = qi * (ND // 4), (qi + 1) * (ND // 4)
                nc.sync.dma_start(
                    out=wq_sb[:, lo * 256: hi * 256].rearrange(
                        "p (c m) -> p c m", c=hi - lo),
                    in_=wqT[lo * 128: hi * 128, :].rearrange(
                        "(c p) m -> p c m", p=128))

            def load_x_block(j, quarter=None):
                if quarter is not None:
                    lo, hi = quarter * 4, (quarter + 1) * 4
                    nc.sync.dma_start(
                        out=x0_tiles[quarter][:].rearrange(
                            "p (c m) -> p c m", c=4),
                        in_=xT[lo * 128: hi * 128,
                               j * 512:(j + 1) * 512].rearrange(
                            "(c p) m -> p c m", p=128))
                    return
                xt = x_tiles[j % 3]
                nc.sync.dma_start(
                    out=xt[:].rearrange("p (c m) -> p c m", c=ND),
                    in_=xT[:, j * 512:(j + 1) * 512].rearrange(
                        "(c p) m -> p c m", p=128))

            x_tiles = [XP.tile([128, ND * 512], bf16, tag="xt", name=f"xt{i}")
                       for i in range(3)]
            x0_tiles = [XP.tile([128, 4 * 512], bf16, tag=f"xq{i}", bufs=1,
                                name=f"xq{i}") for i in range(4)]
            # x_tiles reused round-robin by block index; block 0 uses four
            # independent quarter tiles so the first matmuls only wait on
            # the first quarter's DMA.

            def x_slice(j, dc):
                if j == 0:
                    return x0_tiles[dc // 4][:, (dc % 4) * 512:
                                             (dc % 4 + 1) * 512]
                return x_tiles[j % 3][:, dc * 512:(dc + 1) * 512]

            def x_slice_nat(j, dc, tj):
                if j == 0:
                    t = x0_tiles[dc // 4]
                    o = (dc % 4) * 512 + tj * 128
                    return t[:, o: o + 128]
                t = x_tiles[j % 3]
                o = dc * 512 + tj * 128
                return t[:, o: o + 128]

            # ---------------- pending output-projection queue ----------------
            pending_wo = []
            copy_rr = {"i": 0}

            def emit_wo_unit(unit, dma_per_dq=False, dve_only=False):
                base, tj, dq, rrT_t, osb_t = unit
                tcx0 = base + tj * 128
                wo_ps = PWO.tile([128, 512], f32, tag="wo", name="wo_ps")
                for h in range(HPC):
                    nc.tensor.matmul(
                        wo_ps[:],
                        lhsT=AVT8[:, h * BT + tcx0: h * BT + tcx0 + 128],
                        rhs=wo_sb[:, h * D + dq * 512: h * D + (dq + 1) * 512],
                        start=(h == 0), stop=(h == HPC - 1))
                dst = osb_t[:, dq * 512:(dq + 1) * 512]
                k = copy_rr["i"] % 16
                copy_rr["i"] += 1
                if k % 4 == 0 and not dve_only:
                    nc.scalar.copy(dst, wo_ps[:])
                else:
                    nc.vector.tensor_copy(dst, wo_ps[:])
                if dma_per_dq:
                    nc.sync.dma_start(
                        out=out[tcx0: tcx0 + 128, dq * 512:(dq + 1) * 512],
                        in_=dst)
                elif dq == 3:
                    nc.sync.dma_start(
                        out=out[tcx0: tcx0 + 128, :], in_=osb_t[:])

            tail_mode = {"on": False}

            def pop_wo(n=1, min_keep=0, dma_per_dq=False):
                for _ in range(n):
                    if len(pending_wo) > min_keep:
                        emit_wo_unit(pending_wo.pop(0), dma_per_dq=dma_per_dq,
                                     dve_only=tail_mode["on"])

            # ---------------- projection block ----------------
            def proj_block(j, min_keep=0):
                tcol = slice(j * 512, (j + 1) * 512)
                tab = slice((j * 512) % T, (j * 512) % T + 512)

                def rope(tgt):
                    rswp = RTP.tile([128, 512], bf16, tag="rs")
                    nc.sync.dma_start(out=rswp[0:64, :], in_=tgt[64:128, tcol])
                    nc.sync.dma_start(out=rswp[64:128, :], in_=tgt[0:64, tcol])
                    rtmp = RTP.tile([128, 512], bf16, tag="rt")
                    nc.gpsimd.tensor_mul(rtmp[:], rswp[:], sin_sb[:, tab])
                    nc.gpsimd.tensor_mul(tgt[:, tcol], tgt[:, tcol],
                                         cos_sb[:, tab])
                    nc.gpsimd.tensor_add(tgt[:, tcol], tgt[:, tcol], rtmp[:])

                # q0, q1 chains
                pq = [PPJ.tile([128, 512], f32, tag="pj", name=f"pq{h}")
                      for h in range(HPC)]
                for dc in range(ND):
                    xs = x_slice(j, dc)
                    st, sp = (dc == 0), (dc == ND - 1)
                    for h in range(HPC):
                        nc.tensor.matmul(
                            pq[h][:],
                            lhsT=wq_sb[:, dc * 256 + h * 128:
                                       dc * 256 + (h + 1) * 128],
                            rhs=xs, start=st, stop=sp)
                    if dc % 4 == 3:
                        pop_wo(min_keep=min_keep)
                for h in range(HPC):
                    nc.vector.tensor_copy(QT[h][:, tcol], pq[h][:])
                for h in range(HPC):
                    rope(QT[h])
                pop_wo(min_keep=min_keep)
                # k chain + v natural chains
                pk = PPJ.tile([128, 512], f32, tag="pj", name="pk")
                pv = PPJ.tile([128, 512], f32, tag="pj", name="pv")
                for dc in range(ND):
                    xs = x_slice(j, dc)
                    st, sp = (dc == 0), (dc == ND - 1)
                    nc.tensor.matmul(
                        pk[:], lhsT=wkv_sb[:, dc * 256: dc * 256 + 128],
                        rhs=xs, start=st, stop=sp)
                    if dc % 4 == 3:
                        pop_wo(min_keep=min_keep)
                nc.vector.tensor_copy(KT[:, tcol], pk[:])
                rope(KT)
                for tj in range(4):
                    for dc in range(ND):
                        nc.tensor.matmul(
                            pv[:, tj * 128:(tj + 1) * 128],
                            lhsT=x_slice_nat(j, dc, tj),
                            rhs=wkv_sb[:, dc * 256 + 128: dc * 256 + 256],
                            start=(dc == 0), stop=(dc == ND - 1))
                    pop_wo(min_keep=min_keep)
                nc.vector.tensor_copy(Vn[:, tcol], pv[:])
                pop_wo(min_keep=min_keep)

            # ---------------- attention group ----------------
            def attn_group(b, tsb):
                n_sc = (tsb + 1) * 4
                base = b * T + tsb * 512
                lt_full = PLT.tile([128, 8], f32, tag="lt", name="lt_ps")
                rrT = RRP.tile([128, 8], f32, tag="rrT")
                osb_tiles = [OSBP.tile([128, D], bf16, tag="osb",
                                       name=f"osb{tj}") for tj in range(4)]
                for h in range(HPC):
                    av_ps = PAV.tile([128, 512], f32, tag="av", name="av_ps")
                    pts = []
                    for sc in range(n_sc):
                        sc_rel = sc - tsb * 4
                        c0 = max(sc_rel, 0) * 128
                        nv = slice(c0, 512)
                        st_ps = PST.tile([128, 512], f32, tag="st",
                                         name="st_ps")
                        nc.tensor.matmul(
                            st_ps[:, nv],
                            lhsT=KT[:, b * T + sc * 128:
                                    b * T + (sc + 1) * 128],
                            rhs=QT[h][:, base + c0: base + 512],
                            start=True, stop=True)
                        pt = PTP.tile([128, 512], bf16, tag="pt")
                        pts.append(pt)
                        nc.scalar.activation(
                            pt[:, nv], st_ps[:, nv],
                            mybir.ActivationFunctionType.Exp)
                        if sc_rel >= 0:
                            blk = pt[:, c0:c0 + 128]
                            nc.gpsimd.affine_select(
                                out=blk, in_=blk,
                                compare_op=mybir.AluOpType.is_ge,
                                fill=0.0, base=0, pattern=[[1, 128]],
                                channel_multiplier=-1)
                        nc.tensor.matmul(
                            av_ps[:, nv],
                            lhsT=Vn[:, b * T + sc * 128:
                                    b * T + (sc + 1) * 128],
                            rhs=pt[:, nv],
                            start=(sc == 0), stop=(sc == n_sc - 1))
                        pop_wo()
                    for tj in range(4):
                        last = tsb * 4 + tj
                        for sc in range(last + 1):
                            nc.tensor.matmul(
                                lt_full[:, h * 4 + tj: h * 4 + tj + 1],
                                lhsT=pts[sc][:, tj * 128:(tj + 1) * 128],
                                rhs=ones_col[:],
                                start=(sc == 0), stop=(sc == last))
                    # normalize: rr = 1/l, transposed to rows, broadcast
                    # via outer product, applied in the AVT copy.
                    nc.vector.reciprocal(rrT[:, h * 4: h * 4 + 4],
                                         lt_full[:, h * 4: h * 4 + 4])
                    tr_ps = PST.tile([128, 512], f32, tag="st", name="tr_ps")
                    for tj in range(4):
                        nc.tensor.transpose(
                            tr_ps[0:1, tj * 128:(tj + 1) * 128],
                            rrT[:, h * 4 + tj: h * 4 + tj + 1], ident_f[:])
                    lrow = LRP.tile([1, 512], bf16, tag="lrow")
                    nc.vector.tensor_copy(lrow[:], tr_ps[0:1, 0:512])
                    rr_ps = PST.tile([128, 512], f32, tag="st", name="rr_ps")
                    for tj in range(4):
                        nc.tensor.matmul(
                            rr_ps[:, tj * 128:(tj + 1) * 128],
                            lhsT=ones_row[:],
                            rhs=lrow[0:1, tj * 128:(tj + 1) * 128],
                            start=True, stop=True)
                    rr_sb = RTP.tile([128, 512], f32, tag="rrsb")
                    if (b + tsb + h) % 2 == 0:
                        nc.scalar.copy(rr_sb[:], rr_ps[:])
                    else:
                        nc.vector.tensor_copy(rr_sb[:], rr_ps[:])
                    nc.vector.tensor_mul(AVT8[:, h * BT + base: h * BT + base + 512],
                                         av_ps[:], rr_sb[:])
                for tj in range(4):
                    for dq in range(4):
                        pending_wo.append((base, tj, dq, rrT, osb_tiles[tj]))

            # ---------------- emission schedule ----------------
            load_x_block(0, quarter=0)
            load_wq_quarter(0)
            load_x_block(0, quarter=1)
            load_wq_quarter(1)
            load_x_block(0, quarter=2)
            load_x_block(0, quarter=3)
            load_wq_quarter(2)
            load_wq_quarter(3)
            nc.sync.dma_start(
                out=wkv_sb[:].rearrange("p (c m) -> p c m", c=ND),
                in_=wkvT[:, :].rearrange("(c p) m -> p c m", p=128))
            load_x_block(1)
            nc.sync.dma_start(out=cos_sb[:], in_=cosT[:, :])
            nc.sync.dma_start(out=sin_sb[:], in_=sinT[:, :])
            load_x_block(2)
            nc.sync.dma_start(
                out=wo_sb[:].rearrange("p (c n) -> p c n", c=HPC),
                in_=woT[:, :].rearrange("(c p) n -> p c n", p=128))

            # ramp the PE while the first loads land
            for _ in range(26):
                wps = PST.tile([128, 512], f32, tag="st", name="warm_ps")
                nc.tensor.matmul(wps[:, 0:128], lhsT=warm_sb[:],
                                 rhs=warm_sb[:], start=True, stop=True)

            schedule = [
                ("P", 0), ("P", 1), ("A", 0, 0),
                ("P", 2), ("A", 0, 1),
                ("P", 3), ("A", 0, 2),
                ("P", 4), ("A", 0, 3),
                ("P", 5), ("A", 1, 0),
                ("P", 6), ("A", 1, 1), ("A", 1, 2),
                ("P", 7), ("A", 1, 3),
            ]
            for item in schedule:
                if item[0] == "P":
                    j = item[1]
                    if j + 2 < NBLK and j > 0:
                        load_x_block(j + 2)
                    proj_block(j, min_keep=12 if j == 7 else 0)
                else:
                    attn_group(item[1], item[2])
            pop_wo(len(pending_wo), dma_per_dq=True)

    if split_waits:
        _split_multi_waits(nc, mybir)
    return nc


def _host_inputs(x, wq, wk, wv, wo):
    import ml_dtypes
    bf = ml_dtypes.bfloat16
    f8 = ml_dtypes.float8_e4m3
    xT = np.ascontiguousarray(x.reshape(BT, D).T).astype(bf)
    half = DH // 2
    inv = (1.0 / (ROPE_BASE ** (np.arange(half, dtype=np.float32) / half))
           ).astype(np.float32)
    ang = np.arange(T, dtype=np.float32)[:, None] * inv[None, :]   # (T, 64)
    c = np.cos(ang).T.astype(np.float32)                           # (64, T)
    s = np.sin(ang).T.astype(np.float32)
    cosT = np.ascontiguousarray(np.concatenate([c, c], axis=0)).astype(bf)
    # first half negated: rope rot = [-x2; x1]*sin == [x2; x1]*[-s; s]
    sinT = np.ascontiguousarray(np.concatenate([-s, s], axis=0)).astype(bf)
    rotMT = np.zeros((DH, DH), dtype=np.float32)
    rotMT[np.arange(64), np.arange(64) + 64] = 1.0
    rotMT[np.arange(64) + 64, np.arange(64)] = -1.0
    rotMT = rotMT.astype(bf)
    scale = np.float32(1.0 / np.sqrt(DH))
    in_maps = []
    for core in range(NCORES):
        kvh = core // 2
        wkv = np.concatenate(
            [wk[kvh * DH:(kvh + 1) * DH, :].T,
             wv[kvh * DH:(kvh + 1) * DH, :].T], axis=1)   # (D, 256)
        in_maps.append({
            "xT": xT,
            "wqT": np.ascontiguousarray(
                (wq[core * HPC * DH:(core + 1) * HPC * DH, :] * scale).T
            ).astype(bf),
            "wkvT": np.ascontiguousarray(wkv).astype(bf),
            "woT": np.ascontiguousarray(
                wo[:, core * HPC * DH:(core + 1) * HPC * DH].T).astype(bf),
            "cosT": cosT,
            "sinT": sinT,
            "rotMT": rotMT,
        })
    return in_maps


def kernel(x, wq, wk, wv, wo):
    _ensure_path()
    from concourse.bass_utils import run_bass_kernel_spmd

    x = np.asarray(x, dtype=np.float32)
    wq = np.asarray(wq, dtype=np.float32)
    wk = np.asarray(wk, dtype=np.float32)
    wv = np.asarray(wv, dtype=np.float32)
    wo = np.asarray(wo, dtype=np.float32)

    if "nc" not in _cache:
        _cache["nc"] = _build()
    nc = _cache["nc"]

    in_maps = _host_inputs(x, wq, wk, wv, wo)
    res = run_bass_kernel_spmd(nc, in_maps, list(range(NCORES)))
    acc = res.results[0]["out"].astype(np.float32)
    for cidx in range(1, NCORES):
        acc = acc + res.results[cidx]["out"].astype(np.float32)
    return acc.reshape(B, T, D)
